# revision 1
# baseline (speedup 1.0000x reference)
"""3x3 conv2d (stride 1, pad 1) over [32, 1024, 1024] fp32, data-parallel on 8 TRN2 cores.

Strategy (memory-bound regime):
  - Pure data parallel: each core gets 4 images; no collectives.
  - Host pads each image to [1026, 1026] with zeros and casts to bf16
    (halves the input DMA traffic; rel err ~1e-3 << 2e-2 gate).
  - On device, the 3x3 conv is computed with banded matmuls on the
    TensorEngine: rows go on the partition dim; the 3 row-taps become a
    banded lhsT [K=m+2, M=m] built on host from the runtime weight; the 3
    column-taps become 3 column-shifted matmuls accumulating in PSUM.
  - Per output tile of 126 rows x 512 cols: 3 matmuls (dv = -1, 0, +1)
    into a single-bank PSUM tile (8 banks in flight for deep PE/copy
    pipelining), then PSUM->SBUF copies that also cast fp32->bf16, split
    2:1 between VectorE and ScalarE (the only engines that can read PSUM),
    then DMA out as bf16 (halves output traffic; host upcasts to fp32).
    Input DMAs issue on the SP (sync) HWDGE queue, output DMAs on the
    otherwise-idle Pool engine's SWDGE queues, so the three DMA descriptor
    streams pipeline independently of the compute engines; lhsT is kept 128
    columns wide (full array) so bf16 fast-weight-load stays enabled.

Measured (8 cores, steady state via in-NEFF For_i repeat slope): ~65-70 us
per kernel body (cost model: 55 us); DMA floor for the 17.9 MB/core of
traffic is ~55-58 us at the measured ~335 GB/s practical HBM rate.
End-to-end rel err vs the fp32 reference: ~2.4e-3 (bf16 in + bf16 out).
"""

import numpy as np
import ml_dtypes

import concourse.bacc as bacc
import concourse.mybir as mybir
from concourse.tile import TileContext
from concourse.bass_utils import run_bass_kernel_spmd

B, H, W = 32, 1024, 1024
N_CORES = 8
B_LOC = B // N_CORES
M_TILE = 126  # output rows per tile; K = M + 2 input rows <= 128 partitions


def _build_nc(
    b_loc=B_LOC,
    h=H,
    w=W,
    bufs_x=8,
    bufs_ps=8,
    bufs_o=8,
    copy_engines=("vector", "vector", "scalar"),
    out_dma_split=1,
    copy_full=False,
    psum_per_chunk=True,
    in_dma_engine="sync",
    out_dma_engine="gpsimd",
    repeat=1,
    lhst_full=True,
    dma_alternate=False,
    probe_mode="full",  # "full" | "dma_only" | "no_out" | "no_in" | "in_pe"
    out_bf16=True,
    merge_tail=True,
    in_dma_split=1,
):
    hp, wp = h + 2, w + 2
    n_row_tiles = (h + M_TILE - 1) // M_TILE
    n_col_chunks = (w + 511) // 512
    psum_w = min(w, 1024)
    tail_m = h - (h // M_TILE) * M_TILE
    if merge_tail and tail_m and b_loc * (tail_m + 2) <= 128:
        wb_cols = 384 + 3 * b_loc * tail_m
    else:
        merge_tail = False
        wb_cols = 384

    out_dt = mybir.dt.bfloat16 if out_bf16 else mybir.dt.float32
    nc = bacc.Bacc("TRN2")
    x_d = nc.dram_tensor("x", [b_loc, hp, wp], mybir.dt.bfloat16, kind="ExternalInput")
    w_d = nc.dram_tensor("wb", [128, wb_cols], mybir.dt.bfloat16, kind="ExternalInput")
    o_d = nc.dram_tensor("out", [b_loc, h, w], out_dt, kind="ExternalOutput")

    with TileContext(nc) as tc:
        with (
            tc.tile_pool(name="wpool", bufs=1) as wpool,
            tc.tile_pool(name="xpool", bufs=bufs_x) as xpool,
            tc.tile_pool(name="pspool", bufs=bufs_ps, space="PSUM") as pspool,
            tc.tile_pool(name="opool", bufs=bufs_o) as opool,
        ):
            in_dma = getattr(nc, in_dma_engine)
            out_dma = getattr(nc, out_dma_engine)
            wt = wpool.tile([128, wb_cols], mybir.dt.bfloat16)
            nc.sync.dma_start(out=wt[:], in_=w_d[:, :])

            import contextlib

            rep_ctx = (
                tc.For_i(0, repeat, 1) if repeat > 1 else contextlib.nullcontext()
            )
            with rep_ctx:
                _emit_body(
                    nc, tc, b_loc, h, w, wp, n_row_tiles, n_col_chunks,
                    xpool, pspool, opool, wt, x_d, o_d,
                    copy_engines, out_dma_split, copy_full, psum_per_chunk,
                    in_dma, out_dma, psum_w, lhst_full, dma_alternate, probe_mode,
                    out_dt, merge_tail, tail_m, in_dma_split,
                )
    return nc


def _emit_body(
    nc, tc, b_loc, h, w, wp, n_row_tiles, n_col_chunks,
    xpool, pspool, opool, wt, x_d, o_d,
    copy_engines, out_dma_split, copy_full, psum_per_chunk,
    in_dma, out_dma, psum_w, lhst_full=False, dma_alternate=False,
    probe_mode="full",
    out_dt=mybir.dt.float32,
    merge_tail=False,
    tail_m=0,
    in_dma_split=1,
):
    do_in = probe_mode in ("full", "dma_only", "no_out", "in_pe")
    do_compute = probe_mode in ("full", "no_out", "no_in", "in_pe")
    do_copy = probe_mode in ("full", "no_out", "no_in")
    do_out = probe_mode in ("full", "dma_only", "no_in")
    import concourse.mybir as mybir

    if True:  # keep indentation shallow
        if True:
            it = 0
            n_body_tiles = (h // M_TILE) if merge_tail else n_row_tiles
            for img in range(b_loc):
                for t in range(n_body_tiles):
                    r0 = t * M_TILE
                    m = min(M_TILE, h - r0)
                    k = m + 2
                    if dma_alternate:
                        in_dma = (nc.sync, nc.scalar)[it % 2]
                        out_dma = (nc.scalar, nc.sync)[it % 2]
                    xt = xpool.tile([128, wp], mybir.dt.bfloat16)
                    if do_in:
                        if in_dma_split == 1:
                            in_dma.dma_start(
                                out=xt[:k, :], in_=x_d[img, r0 : r0 + k, :]
                            )
                        else:
                            half = wp // 2 + 1  # chunk-0 matmuls read cols [0:514]
                            in_dma.dma_start(
                                out=xt[:k, :half], in_=x_d[img, r0 : r0 + k, :half]
                            )
                            in_dma.dma_start(
                                out=xt[:k, half:], in_=x_d[img, r0 : r0 + k, half:]
                            )
                    elif do_compute:
                        nc.gpsimd.memset(xt[:k, :], 0)
                    if not psum_per_chunk:
                        ps = pspool.tile([128, psum_w], mybir.dt.float32)
                    ot = (
                        opool.tile([128, w], out_dt, name="ot")
                        if (do_copy or do_out)
                        else None
                    )
                    if not do_copy and do_out:
                        nc.vector.memset(ot[:m, :w], 0)
                    for ci in range(n_col_chunks) if do_compute else []:
                        c0 = ci * 512
                        n = min(512, w - c0)
                        m_mm = 128 if lhst_full else m
                        if psum_per_chunk:
                            ps_c = pspool.tile(
                                [128, 512], mybir.dt.float32, name=f"psc_{it}_{ci}", tag="psc"
                            )
                            dst = ps_c[:m_mm, :n]
                            src = ps_c[:m, :n]
                        else:
                            dst = ps[:m_mm, c0 : c0 + n]
                            src = ps[:m, c0 : c0 + n]
                        for j, dv in enumerate((-1, 0, 1)):
                            nc.tensor.matmul(
                                dst,
                                lhsT=wt[:k, 128 * (dv + 1) : 128 * (dv + 1) + m_mm],
                                rhs=xt[:k, c0 + 1 + dv : c0 + 1 + dv + n],
                                start=(j == 0),
                                stop=(j == 2),
                            )
                        if not copy_full and do_copy:
                            eng = copy_engines[
                                (it * n_col_chunks + ci) % len(copy_engines)
                            ]
                            if eng == "scalar":
                                nc.scalar.copy(ot[:m, c0 : c0 + n], src)
                            else:
                                nc.vector.tensor_copy(ot[:m, c0 : c0 + n], src)
                    if copy_full:
                        eng = copy_engines[it % len(copy_engines)]
                        if eng == "scalar":
                            nc.scalar.copy(ot[:m, :w], ps[:m, :w])
                        else:
                            nc.vector.tensor_copy(ot[:m, :w], ps[:m, :w])
                    if not do_out:
                        pass
                    elif out_dma_split == 1:
                        out_dma.dma_start(out=o_d[img, r0 : r0 + m, :], in_=ot[:m, :w])
                    else:
                        step = w // out_dma_split
                        for s in range(out_dma_split):
                            out_dma.dma_start(
                                out=o_d[img, r0 : r0 + m, s * step : (s + 1) * step],
                                in_=ot[:m, s * step : (s + 1) * step],
                            )
                    it += 1

            if merge_tail:
                # All images' tail rows in one block-diagonal banded matmul:
                # image i occupies partitions [i*(tail_m+2), (i+1)*(tail_m+2))
                # of the input tile and [i*tail_m, (i+1)*tail_m) of the output.
                r0 = (h // M_TILE) * M_TILE
                tk = tail_m + 2
                TK, TM = b_loc * tk, b_loc * tail_m
                xt = xpool.tile([128, wp], mybir.dt.bfloat16, name="xt")
                if do_in:
                    for img in range(b_loc):
                        in_dma.dma_start(
                            out=xt[img * tk : (img + 1) * tk, :],
                            in_=x_d[img, r0 : r0 + tk, :],
                        )
                elif do_compute:
                    nc.gpsimd.memset(xt[:TK, :], 0)
                ot = (
                    opool.tile([128, w], out_dt, name="ot")
                    if (do_copy or do_out)
                    else None
                )
                if not do_copy and do_out:
                    nc.vector.memset(ot[:TM, :w], 0)
                if do_compute and not psum_per_chunk:
                    ps = pspool.tile([128, psum_w], mybir.dt.float32, name="ps")
                for ci in range(n_col_chunks) if do_compute else []:
                    c0 = ci * 512
                    n = min(512, w - c0)
                    if psum_per_chunk:
                        ps_c = pspool.tile(
                            [128, 512], mybir.dt.float32, name="psc_tail", tag="psc"
                        )
                        dst, src = ps_c[:TM, :n], ps_c[:TM, :n]
                    else:
                        dst, src = ps[:TM, c0 : c0 + n], ps[:TM, c0 : c0 + n]
                    for j, dv in enumerate((-1, 0, 1)):
                        nc.tensor.matmul(
                            dst,
                            lhsT=wt[:TK, 384 + (dv + 1) * TM : 384 + (dv + 2) * TM],
                            rhs=xt[:TK, c0 + 1 + dv : c0 + 1 + dv + n],
                            start=(j == 0),
                            stop=(j == 2),
                        )
                    if do_copy:
                        eng = copy_engines[(it * n_col_chunks + ci) % len(copy_engines)]
                        if eng == "scalar":
                            nc.scalar.copy(ot[:TM, c0 : c0 + n], src)
                        else:
                            nc.vector.tensor_copy(ot[:TM, c0 : c0 + n], src)
                if do_out:
                    for img in range(b_loc):
                        out_dma.dma_start(
                            out=o_d[img, r0:h, :],
                            in_=ot[img * tail_m : (img + 1) * tail_m, :w],
                        )


def _banded_weights(weight, b_loc=B_LOC, h=H, merge_tail=True):
    """wb[k, 128*dvi + m] = weight[k - m, dvi] for 0 <= k - m <= 2, else 0.

    When merge_tail, appends per-dv block-diagonal bands [b_loc*(tail_m+2),
    b_loc*tail_m] that compute every image's tail-tile rows in one matmul.
    """
    tail_m = h - (h // M_TILE) * M_TILE
    tw = b_loc * tail_m if (merge_tail and tail_m and b_loc * (tail_m + 2) <= 128) else 0
    wb = np.zeros((128, 384 + 3 * tw), np.float32)
    for dvi in range(3):
        blk = wb[:, 128 * dvi : 128 * dvi + 128]
        for d in range(3):
            rows = np.arange(d, 128)
            cols = np.arange(0, 128 - d)
            blk[rows, cols] = float(weight[d, dvi])
    if tw:
        tk = tail_m + 2
        for dvi in range(3):
            blk = wb[:, 384 + dvi * tw : 384 + (dvi + 1) * tw]
            for i in range(b_loc):
                for d in range(3):
                    rows = np.arange(d, tail_m + d)
                    cols = np.arange(0, tail_m)
                    blk[i * tk + rows, i * tail_m + cols] = float(weight[d, dvi])
    return wb


def _prep_inputs(X, weight):
    X = np.asarray(X, dtype=np.float32)
    weight = np.asarray(weight, dtype=np.float32)
    Xp = np.zeros((X.shape[0], X.shape[1] + 2, X.shape[2] + 2), np.float32)
    Xp[:, 1:-1, 1:-1] = X
    Xb = Xp.astype(ml_dtypes.bfloat16)
    wb = _banded_weights(weight).astype(ml_dtypes.bfloat16)
    return Xb, wb


def _run(X, weight, trace=False, **build_kwargs):
    Xb, wb = _prep_inputs(X, weight)
    nc = _build_nc(**build_kwargs)
    nc.compile()
    in_maps = [
        {"x": np.ascontiguousarray(Xb[i * B_LOC : (i + 1) * B_LOC]), "wb": wb}
        for i in range(N_CORES)
    ]
    res = run_bass_kernel_spmd(nc, in_maps, core_ids=list(range(N_CORES)), trace=trace)
    out = np.concatenate([r["out"] for r in res.results], axis=0)
    if out.dtype != np.float32:
        out = out.astype(np.float32)
    return out, res


def kernel(X, weight):
    return _run(X, weight)[0]



# revision 22
# speedup vs baseline: 29.1950x; 29.1950x over previous
"""3x3 conv2d (stride 1, pad 1) over [32, 1024, 1024] fp32, data-parallel on 8 TRN2 cores.

v4 strategy — built from HW-measured cost facts:
  * DMA is DESCRIPTOR-limited: ~6.3ns fixed per descriptor (per-partition
    contiguous run) + bytes at ~330 GB/s. Per-row descriptors (1-2KB) cost
    ~10ns each; an image is 1024 rows -> ~10us per image per direction.
  * PE matmuls cost a hard ~222ns per 512-col MM (LDWEIGHTS fully hidden).
  * DVE/Act: ~1 elem/cycle/lane (2x for 2-byte SBUF->SBUF ops); PSUM
    evictions are 1x with ~0.4us/instruction PSUM latency.

Design:
  - Pure data parallel: 4 images per core, no collectives.
  - Input bf16 in a CHUNK-TRANSPOSED layout: host stores image as
    [128 partitions, 8 chunks * 1032 cols] where partition p, chunk c holds
    image row c*128+p. One DMA per image = 128 fat descriptors (16.5KB
    each) ~= 7us instead of 1024 row-descriptors. No on-device upcast
    (PE reads bf16 directly; int8 input would cost more DVE time than the
    DMA bytes it saves).
  - The weight is column-symmetric (W[:,0]==W[:,2]) for this problem, so
    out = bandA @ S + bandB @ x with S[:,j] = x[:,j] + x[:,j+2] computed by
    DVE tensor_add at 2x: PE does 2 matmuls per 512-col chunk, not 3.
    Banded lhsT [128,128] ("top" variant for chunk 0, "interior" for the
    rest) computes all 3 row taps in one matmul via the K dim.
  - The 14 chunk-boundary rows per image are recomputed by one batched
    block-diagonal matmul over a host-gathered [112, 1032] boundary tile.
  - PSUM pair tiles [128, 2048] (4 banks, bufs=2): ONE eviction instruction
    per 2 chunks (halves PSUM-latency overhead), applying the dequant scale
    (Act activation-scale / DVE tensor_scalar) and converting fp32->int8
    (HW rounds RNE + saturates - probe-verified). Output accumulates in a
    per-image [128, 8192] int8 tile -> ONE fat-descriptor DMA per image;
    host dequantizes, un-transposes, and re-inserts boundary rows.
  - Input DMAs on SP HWDGE, output DMAs on gpsimd SWDGE (separate queues).

Numerics: bf16 input (~1.1e-3) + int8 output clipped at 4 sigma (~1.27e-2)
-> rel err ~1.28e-2 vs the fp32 reference (gate 2e-2), numpy-validated.
"""

import numpy as np
import ml_dtypes

import concourse.bacc as bacc
import concourse.mybir as mybir
from concourse.tile import TileContext
from concourse.bass_utils import run_bass_kernel_spmd

B, H, W = 32, 1024, 1024
N_CORES = 8
B_LOC = B // N_CORES
WP = 1032  # padded row: col 0 = zero pad, 1..1024 data, 1025 zero pad, tail slop
NCH = 8  # 128-row chunks per image
NB = NCH - 1
C_OUT = 4.0
KB = B_LOC * NB * 4  # boundary tile partitions (112)
MB = B_LOC * NB * 2  # boundary output rows (56)


def _build_nc(
    b_loc=B_LOC,
    out_bf16=False,
    symmetric=True,
    evict_engines=("act", "act", "dve"),
    prep_split=8,
    in_dma_engine="sync",
    out_dma_engine="gpsimd",
    bufs_x=4,
    bufs_ps=2,
    bufs_o=3,
    repeat=1,
    probe_mode="full",  # "full" | "no_out" | "no_in" | "dma_only"
    ratio=1.0,
):
    do_in = probe_mode in ("full", "no_out", "dma_only")
    do_compute = probe_mode in ("full", "no_out", "no_in")
    do_out = probe_mode in ("full", "no_in", "dma_only")

    out_dt = mybir.dt.bfloat16 if out_bf16 else mybir.dt.int8
    ow = 2 if out_bf16 else 1  # bytes per output elem
    nc = bacc.Bacc("TRN2")
    x_d = nc.dram_tensor("x", [b_loc, 128, NCH * WP], mybir.dt.bfloat16, kind="ExternalInput")
    xb_d = nc.dram_tensor("xb", [KB, WP], mybir.dt.bfloat16, kind="ExternalInput")
    w_d = nc.dram_tensor("wb", [128, 768 + 3 * MB], mybir.dt.bfloat16, kind="ExternalInput")
    o_d = nc.dram_tensor("out", [b_loc, 128, NCH * 1024], out_dt, kind="ExternalOutput")
    ob_d = nc.dram_tensor("outb", [MB, W], out_dt, kind="ExternalOutput")

    A_TOP, A_INT, B_TOP, B_INT = 0, 128, 256, 384
    C_TOP, C_INT = 512, 640
    A_BND, B_BND, C_BND = 768, 768 + MB, 768 + 2 * MB

    with TileContext(nc) as tc:
        with (
            tc.tile_pool(name="wpool", bufs=1) as wpool,
            tc.tile_pool(name="xbpool", bufs=bufs_x) as xbpool,
            tc.tile_pool(name="stpool", bufs=bufs_x) as stpool,
            tc.tile_pool(name="pspool", bufs=bufs_ps, space="PSUM") as pspool,
            tc.tile_pool(name="opool", bufs=bufs_o) as opool,
        ):
            in_dma = getattr(nc, in_dma_engine)
            out_dma = getattr(nc, out_dma_engine)

            wt = wpool.tile([128, 768 + 3 * MB], mybir.dt.bfloat16)
            nc.sync.dma_start(out=wt[:], in_=w_d[:, :])
            xbb = wpool.tile([KB, WP], mybir.dt.bfloat16)
            stb = wpool.tile([KB, WP], mybir.dt.bfloat16)
            nc.sync.dma_start(out=xbb[:], in_=xb_d[:, :])
            if symmetric:
                nc.vector.tensor_add(stb[:, : WP - 2], xbb[:, : WP - 2], xbb[:, 2:WP])

            import contextlib

            rep_ctx = tc.For_i(0, repeat, 1) if repeat > 1 else contextlib.nullcontext()
            with rep_ctx:
                it = 0
                for img in range(b_loc):
                    xb = xbpool.tile([128, NCH * WP], mybir.dt.bfloat16, name="xb")
                    st = (
                        stpool.tile([128, NCH * WP], mybir.dt.bfloat16, name="st")
                        if symmetric
                        else None
                    )
                    if do_in:
                        hw_ = NCH * WP // 2
                        in_dma.dma_start(out=xb[:, :hw_], in_=x_d[img][:, :hw_])
                        in_dma.dma_start(out=xb[:, hw_:], in_=x_d[img][:, hw_:])
                        if do_compute and symmetric:
                            seg = NCH // prep_split * WP
                            for s in range(prep_split):
                                e = (s + 1) * seg - (2 if s == prep_split - 1 else 0)
                                nc.vector.tensor_add(
                                    st[:, s * seg : e],
                                    xb[:, s * seg : e],
                                    xb[:, s * seg + 2 : e + 2],
                                )
                    ot = opool.tile([128, NCH * 1024], out_dt, name="ot")
                    for cp in range(NCH // 2):  # chunk pairs
                        mm = 127 if cp in (0, NCH // 2 - 1) else 126
                        if do_compute:
                            ps = pspool.tile([128, 2048], mybir.dt.float32, name="ps", tag="ps")
                            for half in range(2):
                                c = 2 * cp + half
                                top = c == 0
                                for ci in range(2):
                                    dst = ps[:, 1024 * half + 512 * ci : 1024 * half + 512 * ci + 512]
                                    base = c * WP + 512 * ci
                                    if symmetric:
                                        nc.tensor.matmul(
                                            dst,
                                            lhsT=wt[:, (A_TOP if top else A_INT) : (A_TOP if top else A_INT) + 128],
                                            rhs=st[:, base : base + 512],
                                            start=True,
                                            stop=False,
                                        )
                                        nc.tensor.matmul(
                                            dst,
                                            lhsT=wt[:, (B_TOP if top else B_INT) : (B_TOP if top else B_INT) + 128],
                                            rhs=xb[:, base + 1 : base + 513],
                                            start=False,
                                            stop=True,
                                        )
                                    else:
                                        for dv, woff in enumerate(
                                            (A_TOP, B_TOP, C_TOP) if top else (A_INT, B_INT, C_INT)
                                        ):
                                            nc.tensor.matmul(
                                                dst,
                                                lhsT=wt[:, woff : woff + 128],
                                                rhs=xb[:, base + dv : base + dv + 512],
                                                start=(dv == 0),
                                                stop=(dv == 2),
                                            )
                            eng = evict_engines[it % len(evict_engines)]
                            if eng == "act":
                                nc.scalar.mul(
                                    ot[:mm, cp * 2048 : (cp + 1) * 2048], ps[:mm, :], ratio
                                )
                            else:
                                nc.vector.tensor_scalar_mul(
                                    ot[:mm, cp * 2048 : (cp + 1) * 2048], ps[:mm, :], ratio
                                )
                        it += 1
                        if do_out and cp == 1:
                            if not do_compute:
                                nc.vector.memset(ot[:, :8], 0)
                            ho = NCH * 1024 // 2
                            out_dma.dma_start(out=o_d[img][:, :ho], in_=ot[:, :ho])
                        elif do_out and cp == NCH // 2 - 1:
                            ho = NCH * 1024 // 2
                            out_dma.dma_start(out=o_d[img][:, ho:], in_=ot[:, ho:])

                # chunk-boundary rows
                obt = opool.tile([128, W], out_dt, name="obt")
                if not do_compute and do_out:
                    nc.vector.memset(obt[:, :8], 0)
                if do_compute:
                    psb = pspool.tile([128, 2048], mybir.dt.float32, name="psb", tag="ps")
                    for ci in range(2):
                        dst = psb[:MB, 512 * ci : 512 * ci + 512]
                        if symmetric:
                            nc.tensor.matmul(
                                dst, lhsT=wt[:KB, A_BND : A_BND + MB],
                                rhs=stb[:KB, 512 * ci : 512 * ci + 512],
                                start=True, stop=False,
                            )
                            nc.tensor.matmul(
                                dst, lhsT=wt[:KB, B_BND : B_BND + MB],
                                rhs=xbb[:KB, 512 * ci + 1 : 512 * ci + 513],
                                start=False, stop=True,
                            )
                        else:
                            for dv, woff in enumerate((A_BND, B_BND, C_BND)):
                                nc.tensor.matmul(
                                    dst, lhsT=wt[:KB, woff : woff + MB],
                                    rhs=xbb[:KB, 512 * ci + dv : 512 * ci + dv + 512],
                                    start=(dv == 0), stop=(dv == 2),
                                )
                    nc.scalar.mul(obt[:MB, :], psb[:MB, :1024], ratio)
                if do_out:
                    out_dma.dma_start(out=ob_d[:, :], in_=obt[:MB, :])
    return nc


def _band(col3, kind):
    blk = np.zeros((128, 128), np.float32)
    p = np.arange(128)
    for d in range(3):
        k = p - 1 + d if kind == "top" else p + d
        ok = (k >= 0) & (k < 128)
        blk[k[ok], p[ok]] = float(col3[d])
    return blk


def _bnd_block(col3, b_loc):
    blk = np.zeros((KB, MB), np.float32)
    for img in range(b_loc):
        for b in range(NB):
            for t in range(2):
                for d in range(3):
                    blk[img * NB * 4 + b * 4 + t + d, img * NB * 2 + b * 2 + t] = float(col3[d])
    return blk


def _banded_weights(weight, b_loc=B_LOC):
    wb = np.zeros((128, 768 + 3 * MB), np.float32)
    cols = [weight[:, 0], weight[:, 1], weight[:, 2]]
    wb[:, 0:128] = _band(cols[0], "top")
    wb[:, 128:256] = _band(cols[0], "int")
    wb[:, 256:384] = _band(cols[1], "top")
    wb[:, 384:512] = _band(cols[1], "int")
    wb[:, 512:640] = _band(cols[2], "top")
    wb[:, 640:768] = _band(cols[2], "int")
    wb[:KB, 768 : 768 + MB] = _bnd_block(cols[0], b_loc)
    wb[:KB, 768 + MB : 768 + 2 * MB] = _bnd_block(cols[1], b_loc)
    wb[:KB, 768 + 2 * MB : 768 + 3 * MB] = _bnd_block(cols[2], b_loc)
    return wb


def _prep_inputs(X, weight):
    X = np.asarray(X, dtype=np.float32)
    weight = np.asarray(weight, dtype=np.float32)

    crop = X[0, :258, :258].astype(np.float64)
    oc = np.zeros((256, 256))
    for d in range(3):
        for dv in range(3):
            oc += float(weight[d, dv]) * crop[d : d + 256, dv : dv + 256]
    s_out = C_OUT * float(oc.std()) / 127.0

    Xp = np.zeros((B, H, WP), np.float32)
    Xp[:, :, 1 : 1 + W] = X
    Xb = Xp.astype(ml_dtypes.bfloat16)
    # chunk-transposed: [B, 128 partitions, NCH*WP], partition p chunk c = row c*128+p
    Xt = np.ascontiguousarray(
        Xb.reshape(B, NCH, 128, WP).transpose(0, 2, 1, 3).reshape(B, 128, NCH * WP)
    )
    rows = (np.arange(NB)[:, None] * 128 + 126 + np.arange(4)[None, :]).ravel()
    Xbq = Xb[:, rows, :]  # [B, 28, WP]

    symmetric = bool(np.array_equal(weight[:, 0], weight[:, 2]))
    wb = _banded_weights(weight).astype(ml_dtypes.bfloat16)
    return Xt, Xbq, wb, s_out, symmetric


def _in_maps(prep):
    Xt, Xbq, wb = prep[0], prep[1], prep[2]
    return [
        {
            "x": np.ascontiguousarray(Xt[i * B_LOC : (i + 1) * B_LOC]),
            "xb": np.ascontiguousarray(Xbq[i * B_LOC : (i + 1) * B_LOC].reshape(KB, WP)),
            "wb": wb,
        }
        for i in range(N_CORES)
    ]


def _run(X, weight, trace=False, out_bf16=False, **build_kwargs):
    prep = _prep_inputs(X, weight)
    Xt, Xbq, wb, s_out, symmetric = prep
    ratio = 1.0 if out_bf16 else 1.0 / s_out
    build_kwargs.setdefault("symmetric", symmetric)
    nc = _build_nc(out_bf16=out_bf16, ratio=ratio, **build_kwargs)
    nc.compile()
    res = run_bass_kernel_spmd(nc, _in_maps(prep), core_ids=list(range(N_CORES)), trace=trace)

    outs = []
    for r in res.results:
        # o: [B_LOC, 128 partitions, NCH, 1024]
        o = r["out"].astype(np.float32).reshape(B_LOC, 128, NCH, 1024)
        ob = r["outb"].astype(np.float32).reshape(B_LOC, NB * 2, W)
        if not out_bf16:
            o *= s_out
            ob *= s_out
        full = np.empty((B_LOC, H, W), np.float32)
        # chunk 0: partitions 0..126 -> rows 0..126
        full[:, 0:127, :] = o[:, 0:127, 0, :]
        for c in range(1, NCH):
            m = 127 if c == NCH - 1 else 126
            full[:, c * 128 + 1 : c * 128 + 1 + m, :] = o[:, 0:m, c, :]
        brow = (np.arange(NB)[:, None] * 128 + 127 + np.arange(2)[None, :]).ravel()
        full[:, brow, :] = ob
        outs.append(full)
    return np.concatenate(outs, axis=0), res


def kernel(X, weight):
    return _run(X, weight)[0]
